# revision 1
# baseline (speedup 1.0000x reference)
"""CubicPchipKANLayer Trainium2 kernel.

Math: out[b,o] = sum_i PCHIP_interp(x[b,i]; knots y[i,:,o]) + bias[o]

Reformulation: with t = clip((x - D_MIN)/H, 0, K-1), the PCHIP interpolant is
linear over the knot tables:
    out[b,o] = sum_{i,k} phi(t[b,i]-k) * y[i,o,k] + psi(t[b,i]-k) * H*m[i,o,k]
with phi(s) = r^2(3-2r), psi(s) = s*r^2, r = relu(1-|s|);  m = pchip slopes
(functions of the parameter y only, precomputed host-side).  The device
computes a dense (2*D_IN*K x B) weight matrix on-chip from x and contracts it
with the (2*D_IN*K x D_OUT) tables on the PE.

Sharding: contraction-parallel over d_in -- core c owns i in [32c, 32c+32).
Host sums the 8 partial (D_OUT, B) outputs, transposes, adds bias.

Device pipeline, software-pipelined across i-pairs (stage s, pair j):
  s+0  PE  : s = E_j^T @ [t_hi; t_lo; 1]  (128, B) f32 PSUM
  s+1  ACT : ab = Abs(s)                    fp16 SBUF
  s+2  DVE : u = min(ab,1)+0.5              (TSP)
       ACT : w = Square(u - 1.5)            (bias port; w = (min(|s|,1)-1)^2)
  s+3  Pool: P1 = u*w      (= phi/2, pairs with +2y table)
       DVE : P2 = s_f32*w  (= psi,   pairs with H*m table; s read from PSUM)
  s+4  PE  : 4 accumulating matmuls into two (o_half, B) PSUM accumulators.

The schedule keeps every engine ~equally loaded (~1.05us/pair) and the PE
continuously busy so it ramps to its full 2.4 GHz p-state (warm-up matmuls
cover the t-prep and first pipeline fill).

NOTE: this walrus build allows only ONE semaphore wait per instruction; a
post-scheduling pass splits extra waits onto same-engine NoOps.
"""
import sys
sys.path.insert(0, '/opt/trn_rl_repo')
import numpy as np

B, D_IN, D_OUT, K = 512, 256, 256, 64
D_MIN, D_MAX = -2.0, 2.0
H = (D_MAX - D_MIN) / (K - 1)
N_CORES = 8
I_PER = D_IN // N_CORES          # 32 d_in rows per core
NPAIR = I_PER // 2               # 16 i-pairs per core
DEPTH = 4                        # software-pipeline depth (stages)
WARM_PRE = 4                     # warm matmuls before the bc prologue
WARM_FILL = 10                    # warm matmuls between bc prologue and acc_0

_CACHE = {}


def _pchip_hm(y):
    """H * pchip_slopes(y), float64 internally, mirroring reference._pchip_slopes."""
    y = y.astype(np.float64)
    delta = (y[..., 1:] - y[..., :-1]) / H
    d0, d1 = delta[..., :-1], delta[..., 1:]
    denom = d0 + d1
    small = np.abs(denom) < 1e-12
    hm = 2.0 * d0 * d1 / np.where(small, 1.0, denom)
    hm = np.where(small, 0.0, hm)
    m_inner = np.where(d0 * d1 > 0, hm, 0.0)
    m0 = (3.0 * delta[..., 0] - delta[..., 1]) / 2.0
    mN = (3.0 * delta[..., -1] - delta[..., -2]) / 2.0
    m0 = np.where(m0 * delta[..., 0] <= 0, 0.0, m0)
    mN = np.where(mN * delta[..., -1] <= 0, 0.0, mN)
    cond0 = (delta[..., 0] * delta[..., 1] < 0) & (np.abs(m0) > np.abs(3.0 * delta[..., 0]))
    m0 = np.where(cond0, 3.0 * delta[..., 0], m0)
    condN = (delta[..., -1] * delta[..., -2] < 0) & (np.abs(mN) > np.abs(3.0 * delta[..., -1]))
    mN = np.where(condN, 3.0 * delta[..., -1], mN)
    m = np.concatenate([m0[..., None], m_inner, mN[..., None]], axis=-1)
    return (H * m).astype(np.float32)


def _build_tables(y):
    """Per-core rhs tables, shape (N_CORES, 2*K, 2*NPAIR*D_OUT) fp16.

    Table column group (j, h): h=0 -> +2*y rows for pair j, h=1 -> H*m rows.
    Row layout within a group: 64 k-rows of i0 then 64 k-rows of i1.
    """
    hm = _pchip_hm(y)                                       # (d_in, d_out, K)
    y2 = (2.0 * y.astype(np.float64)).astype(np.float32)
    y2_t = np.ascontiguousarray(np.transpose(y2, (0, 2, 1)))  # (d_in, K, d_out)
    hm_t = np.ascontiguousarray(np.transpose(hm, (0, 2, 1)))
    tbl = np.empty((N_CORES, NPAIR, 2, 2, K, D_OUT), np.float32)
    for c in range(N_CORES):
        i0 = c * I_PER
        tbl[c, :, 0] = y2_t[i0:i0 + I_PER].reshape(NPAIR, 2, K, D_OUT)
        tbl[c, :, 1] = hm_t[i0:i0 + I_PER].reshape(NPAIR, 2, K, D_OUT)
    # (c, j, h, half, k, o) -> rows (half,k) x cols (j,h,o)
    tbl = tbl.transpose(0, 3, 4, 1, 2, 5).reshape(N_CORES, 2 * K, 2 * NPAIR * D_OUT)
    return np.ascontiguousarray(tbl.astype(np.float16))


def _build_selector():
    """E (65, NPAIR*128) fp16: per pair j a (65,128) stationary block.
    Rows 0-31 select t_hi rows (1.0 where (p<64, c==2j) or (p>=64, c==2j+1)),
    rows 32-63 repeat the selector for the t_lo rows, row 64 is -(p mod 64)
    (pairs with the ones-row).  All entries are fp16-exact (ints <= 63)."""
    e = np.zeros((65, NPAIR * 128), np.float16)
    for j in range(NPAIR):
        e[2 * j, j * 128:j * 128 + 64] = 1.0
        e[2 * j + 1, j * 128 + 64:(j + 1) * 128] = 1.0
        e[32 + 2 * j, j * 128:j * 128 + 64] = 1.0
        e[32 + 2 * j + 1, j * 128 + 64:(j + 1) * 128] = 1.0
    e[64] = np.tile(-(np.arange(128, dtype=np.float16) % 64), NPAIR)
    return e


def _build_bass():
    import concourse.bass as bass
    import concourse.tile as tile
    from concourse import mybir

    F32 = mybir.dt.float32
    F16 = mybir.dt.float16
    ACTF = mybir.ActivationFunctionType
    ALU = mybir.AluOpType
    TW = 2 * NPAIR * D_OUT            # 8192 table columns

    nc = bass.Bass()
    xt_d = nc.dram_tensor("xt", [32, B], F32, kind="ExternalInput")
    tbl_d = nc.dram_tensor("tbl", [2 * K, TW], F16, kind="ExternalInput")
    e_d = nc.dram_tensor("sel", [65, NPAIR * 128], F16, kind="ExternalInput")
    tc_d = nc.dram_tensor("tcol", [32, 1], F32, kind="ExternalInput")
    out_d = nc.dram_tensor("out", [D_OUT, B], F16, kind="ExternalOutput")

    with tile.TileContext(nc) as tc:
        with tc.tile_pool(name="const", bufs=1) as cpool, \
             tc.tile_pool(name="abp", bufs=3) as abpool, \
             tc.tile_pool(name="up", bufs=3) as upool, \
             tc.tile_pool(name="wp", bufs=3) as wpool, \
             tc.tile_pool(name="p1p", bufs=3) as p1pool, \
             tc.tile_pool(name="p2p", bufs=3) as p2pool, \
             tc.tile_pool(name="res", bufs=1) as respool, \
             tc.tile_pool(name="pacc", bufs=1, space="PSUM") as paccpool, \
             tc.tile_pool(name="pwarm", bufs=1, space="PSUM") as pwarmpool, \
             tc.tile_pool(name="pbc", bufs=5, space="PSUM") as pbcpool:

            e_t = cpool.tile([65, NPAIR * 128], F16)
            n15_t = cpool.tile([128, 1], F32)
            nc.vector.memset(n15_t[:], -1.5)
            actwarm_t = cpool.tile([128, 1], F16)
            # touch the ACT engine immediately so its 1.28us function-table
            # load overlaps the input DMA instead of gating t-prep
            nc.scalar.activation(actwarm_t[:], n15_t[:], ACTF.Abs)
            tc_t = cpool.tile([32, 1], F32)
            xt_t = cpool.tile([32, B], F32)
            warm_t = cpool.tile([128, B], F16)
            nc.gpsimd.memset(warm_t[:], 0.0)
            nc.sync.dma_start(xt_t[:, :B // 2], xt_d[:, :B // 2])
            nc.sync.dma_start(xt_t[:, B // 2:], xt_d[:, B // 2:])
            nc.scalar.dma_start(tc_t[:], tc_d[:])
            ew = NPAIR * 128 // 4
            for p in range(4):
                nc.sync.dma_start(e_t[:, p * ew:(p + 1) * ew],
                                  e_d[:, p * ew:(p + 1) * ew])
            tbl_t = cpool.tile([2 * K, TW], F16)
            for p in range(8):
                w_ = TW // 8
                nc.sync.dma_start(tbl_t[:, p * w_:(p + 1) * w_],
                                  tbl_d[:, p * w_:(p + 1) * w_])

            accT = [paccpool.tile([128, B], F32, tag=f"accT{q}", name=f"accT{q}")
                    for q in range(2)]
            wacc = pwarmpool.tile([128, B], F32, name="wacc")
            for _ in range(WARM_PRE):
                nc.tensor.matmul(wacc[:], warm_t[:, :128], warm_t[:],
                                 start=True, stop=True)

            # t = clip(x/H - D_MIN/H, 0, K-1), column-halved so the second
            # half pipelines behind the first (shorter critical path)
            t_t = cpool.tile([32, B], F32)
            t2_t = cpool.tile([65, B], F16)
            nc.vector.memset(t2_t[64:65, :], 1.0)
            for h in range(2):
                cs = slice(h * (B // 2), (h + 1) * (B // 2))
                nc.scalar.activation(t_t[:, cs], xt_t[:, cs], ACTF.Relu,
                                     bias=tc_t[:], scale=1.0 / H)
                nc.vector.tensor_scalar_min(t_t[:, cs], t_t[:, cs],
                                            float(K - 1))
                nc.scalar.copy(t2_t[0:32, cs], t_t[:, cs])
                nc.vector.tensor_sub(t2_t[32:64, cs], t_t[:, cs],
                                     t2_t[0:32, cs])

            bacc = [None] * NPAIR
            ab = [None] * NPAIR
            u = [None] * NPAIR
            w = [None] * NPAIR
            P1 = [None] * NPAIR
            P2 = [None] * NPAIR

            for j in range(NPAIR + DEPTH):
                # stage 0: broadcast matmul for pair j
                if j < NPAIR:
                    bacc[j] = pbcpool.tile([128, B], F32, tag="bc",
                                           name=f"bacc{j}")
                    nc.tensor.matmul(bacc[j][:],
                                     e_t[:, j * 128:(j + 1) * 128], t2_t[:],
                                     start=True, stop=True)
                if j == DEPTH:
                    for _ in range(WARM_FILL):
                        nc.tensor.matmul(wacc[:], warm_t[:, :128],
                                         warm_t[:], start=True, stop=True)
                # stage 1: |s| PSUM -> SBUF fp16 for pair j-1
                a = j - 1
                if 0 <= a < NPAIR:
                    ab[a] = abpool.tile([128, B], F16, tag="ab",
                                        name=f"ab{a}")
                    nc.scalar.activation(ab[a][:], bacc[a][:], ACTF.Abs)
                # stage 2: clamp / shift (single-scalar TSPs run at the DVE
                # fast rate; the dual-op form falls back to 1x), then square
                b = j - 2
                if 0 <= b < NPAIR:
                    u[b] = upool.tile([128, B], F16, tag="u", name=f"u{b}")
                    nc.vector.tensor_scalar(u[b][:], ab[b][:], 1.0, 0.5,
                                            op0=ALU.min, op1=ALU.add)
                    w[b] = wpool.tile([128, B], F16, tag="w", name=f"w{b}")
                    nc.scalar.activation(w[b][:], u[b][:], ACTF.Square,
                                         bias=n15_t[:])
                # stage 3: weight planes for pair j-3
                c = j - 3
                if 0 <= c < NPAIR:
                    P2[c] = p2pool.tile([128, B], F16, tag="P2",
                                        name=f"P2_{c}")
                    nc.vector.tensor_mul(P2[c][:], bacc[c][:], w[c][:])
                    P1[c] = p1pool.tile([128, B], F16, tag="P1",
                                        name=f"P1_{c}")
                    eng = nc.vector if c == NPAIR - 1 else nc.gpsimd
                    eng.tensor_mul(P1[c][:], u[c][:], w[c][:])
                # stage 4: accumulating matmuls for pair j-4
                d = j - DEPTH
                if 0 <= d < NPAIR:
                    for hh, w_t in ((1, P2[d]), (0, P1[d])):
                        base = (d * 2 + hh) * D_OUT
                        for q in range(2):
                            nc.tensor.matmul(
                                accT[q][:],
                                tbl_t[:, base + q * 128: base + (q + 1) * 128],
                                w_t[:],
                                start=(d == 0 and hh == 1),
                                stop=(d == NPAIR - 1 and hh == 0))
                    if d < NPAIR - 1:
                        # tiny keep-alive matmuls: absorb inter-pair gaps so
                        # the PE p-state never drops out of full speed
                        for _ in range(2):
                            nc.tensor.matmul(wacc[:, :32], warm_t[:, :128],
                                             warm_t[:, :32],
                                             start=True, stop=True)

            for q in range(2):
                o_t = respool.tile([128, B], F16, tag=f"o{q}", name=f"o{q}")
                for hh in range(2):
                    cols = slice(hh * (B // 2), (hh + 1) * (B // 2))
                    if q == 0:
                        nc.scalar.copy(o_t[:, cols], accT[q][:, cols])
                    else:
                        nc.vector.tensor_copy(o_t[:, cols], accT[q][:, cols])
                    dq = nc.sync if hh == 0 else nc.scalar
                    dq.dma_start(out_d[q * 128:(q + 1) * 128, cols],
                                 o_t[:, cols])

    return nc


def _split_multiwaits(nc):
    """Walrus in this build allows one semaphore wait per instruction.  Tile
    sometimes emits several; split the extras onto same-engine NoOps inserted
    immediately before the instruction (queue order preserves semantics)."""
    from concourse import mybir

    fix_id = 0
    for f in nc.m.functions:
        for blk in f.blocks:
            insts = blk.instructions
            out, changed = [], False
            for ins in insts:
                si = getattr(ins, "sync_info", None)
                waits = list(si.on_wait) if si and si.on_wait else []
                if len(waits) > 1:
                    for wv in waits[:-1]:
                        nop = mybir.InstNoOp(name=f"I-fixw{fix_id}",
                                             engine=ins.engine)
                        fix_id += 1
                        nop.sync_info = mybir.SyncInfo(on_wait=[wv], on_update=[])
                        out.append(nop)
                    ins.sync_info = mybir.SyncInfo(
                        on_wait=[waits[-1]], on_update=list(si.on_update))
                    changed = True
                out.append(ins)
            if changed:
                blk.instructions = out


def _get_compiled():
    if "nc" not in _CACHE:
        nc = _build_bass()
        _split_multiwaits(nc)
        _CACHE["nc"] = nc
    return _CACHE["nc"]


def _run(x, y, bias, trace=False):
    from concourse.bass_utils import run_bass_kernel_spmd

    x = np.asarray(x, np.float32)
    y = np.asarray(y, np.float32)
    bias = np.asarray(bias, np.float32)

    nc = _get_compiled()

    xs = np.ascontiguousarray(x.T)                     # (d_in, B)
    tbl = _build_tables(y)                             # (8, 128, 8192) fp16
    e_np = _build_selector()
    tc_np = np.full((32, 1), -D_MIN / H, np.float32)

    in_maps = []
    for c in range(N_CORES):
        xt = np.ascontiguousarray(xs[c * I_PER:(c + 1) * I_PER])
        in_maps.append({
            "xt": xt,
            "tbl": tbl[c],
            "sel": e_np,
            "tcol": tc_np,
        })
    res = run_bass_kernel_spmd(nc, in_maps, core_ids=list(range(N_CORES)),
                               trace=trace)
    partialT = np.stack([res.results[c]["out"] for c in range(N_CORES)])
    out = partialT.astype(np.float64).sum(axis=0).T + bias.astype(np.float64)
    return out.astype(np.float32), res


def kernel(x, y, bias):
    out, _ = _run(x, y, bias)
    return out

